# revision 25
# baseline (speedup 1.0000x reference)
"""TRN2 Bass kernel for nn_MoE_19825569038534 (moe_routing).

3-layer MoE network: per layer a cosine router (top-4 of 8 experts,
softmax gate) + dense expert mix, LN+relu between layers.
Returns (h_emb, h, aux_loss) like the jax reference.

Strategy: data-parallel over tokens (8192 -> 8 cores x 1024). On-chip
activations are feature-major (hT [d, n]) so expert matmuls stream with
weights stationary and layers chain without activation transposes.
Expert sum uses gate PRE-scaling: out = sum_e (gate_e * h) @ W_e,
accumulated in PSUM/SBUF; gate rows are broadcast across partitions with
K=1 ones-matmuls.

Precision plan (routing top-k flips are the accuracy hazard: a flipped
token is ~0.6 absmax rel err): fp32 for anything that feeds a later
router (layers 0/1 experts, selection matmul h@ (P@s), LN stats),
float32r (full-speed, ~1.4e-4) for value-only paths (layer-2 experts,
router norm projection). Router temperature, sim normalization, P@s
fold, and the x2 layer temp are folded on the host.
"""

import numpy as np

import concourse.bacc as bacc
import concourse.mybir as mybir
from concourse import tile
from concourse.bass_utils import run_bass_kernel_spmd

F32 = mybir.dt.float32
F32R = mybir.dt.float32r
F16 = mybir.dt.float16
AF = mybir.ActivationFunctionType
OP = mybir.AluOpType

# ---- problem dims (hardcoded) ----
N_TOK = 8192
NCORES = 8
N = N_TOK // NCORES          # tokens per core
D_IN, D_HID, D_OUT = 512, 1024, 512
E = 8
RK = 4                       # router k = K+1
LOG100 = float(np.log(100.0))
LB_COEF = 0.01
LN_EPS = 1e-5
NB = N // 512                # 512-token free-dim blocks

# (di, do, has_ln, expert_mode)  mode: "f16x2" (fp32-grade, 3 passes),
# "f32r" (value-only precision, 1 pass), "f32" (native, 4 passes)
LAYERS = [
    (D_IN, D_HID, True, "f16x2"),
    (D_HID, D_HID, True, "f16x2"),
    (D_HID, D_OUT, False, "f16"),
]


def _build():
    nc = bacc.Bacc(None, target_bir_lowering=False)

    # ---------------- DRAM parameters ----------------
    xt_d = nc.declare_dram_parameter("xt", [D_IN // 128, 128, N], F32, isOutput=False)
    ident_d = nc.declare_dram_parameter("ident", [128, 128], F32, isOutput=False)
    sel_d = nc.declare_dram_parameter("sel", [E, E * 128], F32, isOutput=False)
    w_d, bf_d, ps_d, bs_d, pw_d, pb_d, gam_d, bet_d = {}, {}, {}, {}, {}, {}, {}, {}
    for li, (di, do, ln, ln_mode) in enumerate(LAYERS):
        KT, MT = di // 128, do // 128
        if ln_mode == "f16x2":
            w_d[li] = (
                nc.declare_dram_parameter(f"w{li}h", [E, MT, 128, KT, 128], F16, isOutput=False),
                nc.declare_dram_parameter(f"w{li}l", [E, MT, 128, KT, 128], F16, isOutput=False),
            )
        elif ln_mode == "f16":
            w_d[li] = nc.declare_dram_parameter(f"w{li}h", [E, MT, 128, KT, 128], F16, isOutput=False)
        else:
            w_d[li] = nc.declare_dram_parameter(f"w{li}", [E, MT, 128, KT, 128], F32, isOutput=False)
        bf_d[li] = nc.declare_dram_parameter(f"bf{li}", [E, do], F32, isOutput=False)
        ps_d[li] = nc.declare_dram_parameter(f"psf{li}", [128, KT, 8], F32, isOutput=False)
        bs_d[li] = nc.declare_dram_parameter(f"bsf{li}", [1, 8], F32, isOutput=False)
        pw_d[li] = nc.declare_dram_parameter(f"pw{li}", [D_HID // 128, 128, KT, 128], F16, isOutput=False)
        pb_d[li] = nc.declare_dram_parameter(f"pb{li}", [128, D_HID // 128], F32, isOutput=False)
        if ln:
            gam_d[li] = nc.declare_dram_parameter(f"gam{li}", [128, do // 128], F32, isOutput=False)
            bet_d[li] = nc.declare_dram_parameter(f"bet{li}", [128, do // 128], F32, isOutput=False)
    hemb_d = nc.declare_dram_parameter("hembT", [D_HID // 128, 128, N], F32, isOutput=True)
    ht_d = nc.declare_dram_parameter("hT", [D_OUT // 128, 128, N], F32, isOutput=True)
    il_d = nc.declare_dram_parameter("il", [1, 48], F32, isOutput=True)

    with tile.TileContext(nc) as tc:
        cpool = tc.alloc_tile_pool(name="consts", bufs=1)
        hpool = tc.alloc_tile_pool(name="hpool", bufs=1)
        wpool = tc.alloc_tile_pool(name="wpool", bufs=1)
        psum = tc.alloc_tile_pool(name="psum", bufs=1, space="PSUM")
        dpool = tc.alloc_tile_pool(name="dram", bufs=2, space="DRAM")

        # ---------------- constants ----------------
        ident = cpool.tile([128, 128], F32, tag="ident")
        nc.sync.dma_start(ident[:], ident_d[:])
        ones_row = cpool.tile([1, 128], F32, tag="ones_row")   # K=1 lhsT
        nc.vector.memset(ones_row[:], 1.0)
        ones_col = cpool.tile([128, 1], F32, tag="ones_col")   # partition-sum lhsT
        nc.vector.memset(ones_col[:], 1.0)
        ones_col_r = cpool.tile([128, 1], F32R, tag="ones_col_r")
        nc.vector.tensor_copy(ones_col_r[:], ones_col[:])
        sel = cpool.tile([E, E * 128], F32, tag="sel")         # row-e selector lhsT
        nc.sync.dma_start(sel[:], sel_d[:])
        eps1 = cpool.tile([1, 1], F32, tag="eps1")
        nc.vector.memset(eps1[:], LN_EPS)

        ps_sb, bs_sb, bf_sb, pb_sb, gam_sb, bet_sb = {}, {}, {}, {}, {}, {}
        for li, (di, do, ln, _) in enumerate(LAYERS):
            KT = di // 128
            ps_sb[li] = cpool.tile([128, KT, 8], F32, tag=f"psf{li}", name=f"psf{li}")
            nc.sync.dma_start(ps_sb[li][:], ps_d[li][:])
            bs_sb[li] = cpool.tile([1, 8], F32, tag=f"bsf{li}", name=f"bsf{li}")
            nc.sync.dma_start(bs_sb[li][:], bs_d[li][:])
            bf_sb[li] = cpool.tile([E, do], F32, tag=f"bff{li}", name=f"bff{li}")
            nc.sync.dma_start(bf_sb[li][:], bf_d[li][:])
            pb_sb[li] = cpool.tile([128, D_HID // 128], F32, tag=f"pbf{li}", name=f"pbf{li}")
            nc.sync.dma_start(pb_sb[li][:], pb_d[li][:])
            if ln:
                gam_sb[li] = cpool.tile([128, do // 128], F32, tag=f"gamf{li}", name=f"gamf{li}")
                nc.sync.dma_start(gam_sb[li][:], gam_d[li][:])
                bet_sb[li] = cpool.tile([128, do // 128], F32, tag=f"betf{li}", name=f"betf{li}")
                nc.sync.dma_start(bet_sb[li][:], bet_d[li][:])

        il_sb = cpool.tile([1, 48], F32, tag="il_sb")

        # ---------------- input ----------------
        h = []
        for k in range(D_IN // 128):
            t = hpool.tile([128, N], F32, tag="h", bufs=8)
            nc.sync.dma_start(t[:], xt_d[k])
            h.append(t)

        # ---------------- layers ----------------
        for li, (di, do, use_ln, emode) in enumerate(LAYERS):
            KT, MT = di // 128, do // 128
            PT = D_HID // 128  # proj width tiles (norm path)

            # --- layer-wide f16 split of h: experts and norm-proj read these ---
            hhi, hlo = [], []
            for k in range(KT):
                hhi.append(wpool.tile([128, N], F16, tag="hgh", bufs=16, name=f"hhi{k}"))
                if emode == "f16x2":
                    hlo.append(wpool.tile([128, N], F16, tag="hgl", bufs=16, name=f"hlo{k}"))
            for b in range(NB):
                sl = slice(b * 512, (b + 1) * 512)
                for k in range(KT):
                    nc.vector.tensor_copy(hhi[k][:, sl], h[k][:, sl])
                    if emode == "f16x2":
                        nc.vector.tensor_sub(hlo[k][:, sl], h[k][:, sl], hhi[k][:, sl])

            # --- norm projection: S2 = ||h @ P + pb||^2 per token (f16) ---
            s2row = wpool.tile([1, N], F32, tag="rowsb", bufs=4)
            for b in range(NB):
                sl = slice(b * 512, (b + 1) * 512)
                hrb = [hhi[k][:, sl] for k in range(KT)]
                ps_s2 = psum.tile([1, 512], F32, tag="row", bufs=1)
                for m in range(PT):
                    pwm = wpool.tile([128, KT, 128], F16, tag="pwm", bufs=2)
                    nc.sync.dma_start(pwm[:], pw_d[li][m])
                    pt = psum.tile([128, 512], F32, tag="big", bufs=5)
                    for k in range(KT):
                        nc.tensor.matmul(pt[:], pwm[:, k, :],
                                         hrb[k], start=(k == 0), stop=(k == KT - 1))
                    sq = wpool.tile([128, 512], F32R, tag="sqr", bufs=2)
                    nc.scalar.activation(sq[:], pt[:], AF.Square, bias=pb_sb[li][:, m:m + 1])
                    nc.tensor.matmul(ps_s2[:], ones_col_r[:], sq[:],
                                     start=(m == 0), stop=(m == PT - 1))
                nc.vector.tensor_copy(s2row[:, sl], ps_s2[:])
            # --- selection numerator (token-major), fp32 ---
            numer = []
            for t in range(N // 128):
                pn = psum.tile([128, 8], F32, tag="small", bufs=2)
                nc.tensor.matmul(pn[:], ones_row[:], bs_sb[li][:], start=True, stop=False)
                for k in range(KT):
                    nc.tensor.matmul(pn[:], h[k][:, t * 128:(t + 1) * 128],
                                     ps_sb[li][:, k, :], start=False, stop=(k == KT - 1))
                ns = wpool.tile([128, 8], F32, tag="numer", bufs=8)
                nc.vector.tensor_copy(ns[:], pn[:])
                numer.append(ns)

            # keep-warm: tiny PE ops dep'd on the serial rs2 chain so HAM
            # stays at K=8/8 through the PE-idle router phase
            pkw = psum.tile([128, 8], F32, tag="small", bufs=2, name=f"pkw{li}")
            nc.tensor.matmul(pkw[:], ones_row[:], s2row[:, 0:8], start=True, stop=True)

            # rs2 = 1/sqrt(S2) = 1/||t|| (temp already folded into PSfold)
            scr = dpool.tile([1, N], F32, tag="scr")
            nc.sync.dma_start(scr[:], s2row[:])
            rs2a = wpool.tile([128, N // 128], F32, tag="rs2a", bufs=1)
            nc.sync.dma_start(rs2a[:], scr.rearrange("o (t p) -> (o p) t", p=128))
            rs2b = wpool.tile([128, N // 128], F32, tag="rs2b", bufs=1)
            nc.scalar.activation(rs2b[:], rs2a[:], AF.Sqrt)
            rs2 = wpool.tile([128, N // 128], F32, tag="rs2c", bufs=1)
            nc.vector.reciprocal(rs2[:], rs2b[:])
            nc.tensor.matmul(pkw[0:1, :], ones_col[:], rs2[:], start=True, stop=True)

            # --- top-4 mask, softmax gate (token-major) ---
            gate = []
            ps_il = psum.tile([1, 16], F32, tag="row", bufs=1)
            for t in range(N // 128):
                m8 = wpool.tile([128, 8], F32, tag="m8", bufs=2)
                nc.vector.max(m8[:], numer[t][:])
                pm = wpool.tile([128, 16], F32, tag="pm", bufs=2)
                nc.vector.tensor_scalar(pm[:, 8:16], numer[t][:], m8[:, 3:4], None, op0=OP.is_ge)
                se = wpool.tile([128, 1], F32, tag="se", bufs=2)
                nc.scalar.activation(pm[:, 0:8], numer[t][:], AF.Exp,
                                     scale=rs2[:, t:t + 1], accum_out=se[:])
                re = wpool.tile([128, 1], F32, tag="re", bufs=2)
                nc.vector.reciprocal(re[:], se[:])
                nc.vector.tensor_scalar_mul(pm[:, 0:8], pm[:, 0:8], re[:])
                g = wpool.tile([128, 8], F32, tag="gate", bufs=8)
                nc.vector.tensor_tensor(g[:], pm[:, 0:8], pm[:, 8:16], op=OP.mult)
                gate.append(g)
                nc.tensor.matmul(ps_il[:], ones_col[:], pm[:],
                                 start=(t == 0), stop=(t == N // 128 - 1))
                nc.tensor.matmul(pkw[0:1, :], ones_col[:], g[:], start=True, stop=True)
            nc.vector.tensor_copy(il_sb[:, li * 16:(li + 1) * 16], ps_il[:])

            # --- gate transpose: gT [E, N] ---
            gT = wpool.tile([E, N], F32, tag="gT", bufs=2)
            for t in range(N // 128):
                pgt = psum.tile([8, 128], F32, tag="small", bufs=2)
                nc.tensor.transpose(pgt[:], gate[t][:], ident[:])
                nc.vector.tensor_copy(gT[:, t * 128:(t + 1) * 128], pgt[:])

            # --- experts: out[o, t] = sum_e (gate_e h) @ W_e + gate @ bfold ---
            outs = [hpool.tile([128, N], F32, tag="out", bufs=8, name=f"out{_m}") for _m in range(MT)]

            outs = [hpool.tile([128, N], F32, tag="out", bufs=8, name=f"out{_m}") for _m in range(MT)]
            # init outs with the gate-weighted bias sum_e gate_e*b_e
            for m in range(MT):
                for b in range(NB):
                    sl = slice(b * 512, (b + 1) * 512)
                    pb2 = psum.tile([128, 512], F32, tag="big", bufs=5, name="pb2")
                    nc.tensor.matmul(pb2[:], bf_sb[li][:, m * 128:(m + 1) * 128],
                                     gT[:, sl], start=True, stop=True)
                    nc.vector.tensor_copy(outs[m][:, sl], pb2[:])
            # experts: psum holds ungated h@W_e; gate applied on the psum
            # output (gate commutes with the contraction), accumulated in SBUF
            for e in range(E):
                bcss = []
                for b in range(NB):
                    sl = slice(b * 512, (b + 1) * 512)
                    pbc = psum.tile([128, 512], F32, tag="big", bufs=5, name="pbc")
                    nc.tensor.matmul(pbc[:], sel[:, e * 128:(e + 1) * 128], gT[:, sl],
                                     start=True, stop=True)
                    bcs = wpool.tile([128, 512], F32, tag="bcse", bufs=4, name="bcs")
                    nc.vector.tensor_copy(bcs[:], pbc[:])
                    bcss.append(bcs)
                for m in range(MT):
                    if emode == "f16x2":
                        whi = wpool.tile([128, KT, 128], F16, tag="wh", bufs=3)
                        nc.sync.dma_start(whi[:], w_d[li][0][e, m])
                        wlo = wpool.tile([128, KT, 128], F16, tag="wl", bufs=2)
                        nc.sync.dma_start(wlo[:], w_d[li][1][e, m])
                    else:
                        whi = wpool.tile([128, KT, 128], F16, tag="wh", bufs=3)
                        nc.sync.dma_start(whi[:], w_d[li][e, m])
                    for b in range(NB):
                        sl = slice(b * 512, (b + 1) * 512)
                        po = psum.tile([128, 512], F32, tag="big", bufs=5)
                        if emode == "f16x2":
                            passes = [(whi, hhi), (whi, hlo), (wlo, hhi)]
                        else:
                            passes = [(whi, hhi)]
                        np_ = len(passes)
                        for pi, (wt, ht) in enumerate(passes):
                            for k in range(KT):
                                nc.tensor.matmul(
                                    po[:], wt[:, k, :], ht[k][:, sl],
                                    start=(pi == 0 and k == 0),
                                    stop=(pi == np_ - 1 and k == KT - 1))
                        gy = wpool.tile([128, 512], F32, tag="u", bufs=2, name="gy")
                        nc.vector.tensor_tensor(gy[:], po[:], bcss[b][:], op=OP.mult)
                        nc.vector.tensor_tensor(outs[m][:, sl], outs[m][:, sl], gy[:], op=OP.add)

            # --- layer epilogue ---
            if li == 0:
                for m in range(MT):
                    nc.sync.dma_start(hemb_d[m], outs[m][:])
            if li == len(LAYERS) - 1:
                for m in range(MT):
                    nc.sync.dma_start(ht_d[m], outs[m][:])

            if use_ln:
                # S1/S2 over features (partitions), fp32
                s1row_ln = wpool.tile([1, N], F32, tag="rowsb", bufs=4)
                s2row_ln = wpool.tile([1, N], F32, tag="rowsb", bufs=4)
                for b in range(NB):
                    sl = slice(b * 512, (b + 1) * 512)
                    p1 = psum.tile([1, 512], F32, tag="row", bufs=1)
                    for m in range(MT):
                        nc.tensor.matmul(p1[:], ones_col[:], outs[m][:, sl],
                                         start=(m == 0), stop=(m == MT - 1))
                    nc.vector.tensor_copy(s1row_ln[:, sl], p1[:])
                for b in range(NB):
                    sl = slice(b * 512, (b + 1) * 512)
                    p2 = psum.tile([1, 512], F32, tag="row", bufs=1)
                    for m in range(MT):
                        sq = wpool.tile([128, 512], F32R, tag="sqr", bufs=2)
                        nc.scalar.activation(sq[:], outs[m][:, sl], AF.Square)
                        nc.tensor.matmul(p2[:], ones_col_r[:], sq[:],
                                         start=(m == 0), stop=(m == MT - 1))
                    nc.vector.tensor_copy(s2row_ln[:, sl], p2[:])
                mu = wpool.tile([1, N], F32, tag="rowsb", bufs=4)
                nc.vector.tensor_scalar_mul(mu[:], s1row_ln[:], 1.0 / do)
                v1 = wpool.tile([1, N], F32, tag="rowsb", bufs=4)
                nc.vector.tensor_scalar_mul(v1[:], s2row_ln[:], 1.0 / do)
                v2 = wpool.tile([1, N], F32, tag="rowsb", bufs=4)
                nc.vector.tensor_tensor(v2[:], mu[:], mu[:], op=OP.mult)
                var = wpool.tile([1, N], F32, tag="rowsb", bufs=4)
                nc.vector.tensor_sub(var[:], v1[:], v2[:])
                sd = wpool.tile([1, N], F32, tag="rowsb", bufs=4)
                nc.scalar.activation(sd[:], var[:], AF.Sqrt, bias=eps1[:])
                rstd = wpool.tile([1, N], F32, tag="rowsb", bufs=4)
                nc.vector.reciprocal(rstd[:], sd[:])
                nc.tensor.matmul(pkw[:], ones_row[:], mu[:, 0:8], start=True, stop=True)
                nc.tensor.matmul(pkw[:], ones_row[:], rstd[:, 0:8], start=True, stop=True)

                hn = [hpool.tile([128, N], F32, tag="h", bufs=8, name=f"hn{_m}") for _m in range(MT)]
                for b in range(NB):
                    sl = slice(b * 512, (b + 1) * 512)
                    pmu = psum.tile([128, 512], F32, tag="big", bufs=5)
                    nc.tensor.matmul(pmu[:], ones_row[:], mu[:, sl], start=True, stop=True)
                    mus = wpool.tile([128, 512], F32, tag="bcs", bufs=2)
                    nc.vector.tensor_copy(mus[:], pmu[:])
                    prs = psum.tile([128, 512], F32, tag="big", bufs=5)
                    nc.tensor.matmul(prs[:], ones_row[:], rstd[:, sl], start=True, stop=True)
                    rss = wpool.tile([128, 512], F32, tag="bcs", bufs=2)
                    nc.vector.tensor_copy(rss[:], prs[:])
                    for m in range(MT):
                        u = wpool.tile([128, 512], F32, tag="u", bufs=2)
                        nc.vector.tensor_sub(u[:], outs[m][:, sl], mus[:])
                        w2 = wpool.tile([128, 512], F32, tag="u", bufs=2)
                        nc.vector.tensor_tensor(w2[:], u[:], rss[:], op=OP.mult)
                        nc.scalar.activation(hn[m][:, sl], w2[:], AF.Relu,
                                             scale=gam_sb[li][:, m:m + 1],
                                             bias=bet_sb[li][:, m:m + 1])
                        nc.tensor.matmul(pkw[0:1, :], ones_col[:], hn[m][:, sl.start:sl.start + 8],
                                         start=True, stop=True)
                h = hn

        nc.sync.dma_start(il_d[:], il_sb[:])

        for _p in (dpool, psum, wpool, hpool, cpool):
            _p.release()

    nc.compile()
    return nc


_NC = None


def _get_nc():
    global _NC
    if _NC is None:
        _NC = _build()
    return _NC


def _np(a):
    return np.asarray(a, dtype=np.float32)


def host_prep(x, params):
    """Fold params on the host; build per-core in_maps."""
    x = _np(x)
    common = {"ident": np.eye(128, dtype=np.float32)}
    selm = np.zeros((E, E * 128), np.float32)
    for e in range(E):
        selm[e, e * 128:(e + 1) * 128] = 1.0
    common["sel"] = selm
    for li, (di, do, ln, _mode) in enumerate(LAYERS):
        lp = params["layers"][li]
        KT, MT = di // 128, do // 128
        proj_W = np.asarray(lp["proj_W"], np.float64)
        proj_b = np.asarray(lp["proj_b"], np.float64)
        sim = np.asarray(lp["sim"], np.float64)
        rtemp = float(np.exp(min(float(np.asarray(lp["router_temp"]).reshape(-1)[0]), LOG100)))
        ltemp = float(np.exp(min(float(np.asarray(lp["layer_temp"]).reshape(-1)[0]), LOG100)))
        s = sim / np.maximum(np.linalg.norm(sim, axis=0, keepdims=True), 1e-12)
        PS = (proj_W @ s) * rtemp                      # [di, 8]
        bS = (proj_b @ s) * rtemp                      # [8]
        Wf = _np(lp["expert_W"]) * np.float32(ltemp)   # [E, di, do]
        bf = _np(lp["expert_b"]) * np.float32(ltemp)   # [E, do]
        # w layout [E, MT, 128(di_p), KT, 128(do_q)]
        w = np.ascontiguousarray(
            Wf.reshape(E, KT, 128, MT, 128).transpose(0, 3, 2, 1, 4))
        if LAYERS[li][3] == "f16x2":
            whi = w.astype(np.float16)
            wlo = (w - whi.astype(np.float32)).astype(np.float16)
            common[f"w{li}h"] = whi
            common[f"w{li}l"] = wlo
        elif LAYERS[li][3] == "f16":
            common[f"w{li}h"] = w.astype(np.float16)
        else:
            common[f"w{li}"] = w
        common[f"bf{li}"] = np.ascontiguousarray(bf)
        common[f"psf{li}"] = np.ascontiguousarray(
            PS.astype(np.float32).reshape(KT, 128, 8).transpose(1, 0, 2))
        common[f"bsf{li}"] = bS.astype(np.float32).reshape(1, 8)
        common[f"pw{li}"] = np.ascontiguousarray(
            _np(lp["proj_W"]).reshape(KT, 128, D_HID // 128, 128)
            .transpose(2, 1, 0, 3)).astype(np.float16)
        common[f"pb{li}"] = np.ascontiguousarray(
            _np(lp["proj_b"]).reshape(D_HID // 128, 128).T)
        if ln:
            lnp = params["ln"][li]
            common[f"gam{li}"] = np.ascontiguousarray(_np(lnp["scale"]).reshape(MT, 128).T)
            common[f"bet{li}"] = np.ascontiguousarray(_np(lnp["bias"]).reshape(MT, 128).T)

    in_maps = []
    for c in range(NCORES):
        xs = x[c * N:(c + 1) * N]                      # [N, D_IN]
        xt = np.ascontiguousarray(xs.T.reshape(D_IN // 128, 128, N))
        m = dict(common)
        m["xt"] = xt
        in_maps.append(m)
    return in_maps


def _cv_squared(v):
    v = np.asarray(v, np.float32)
    return np.var(v, ddof=1) / (np.mean(v) ** 2 + np.float32(1e-10))


def run_device(x, params, trace=False):
    nc = _get_nc()
    in_maps = host_prep(x, params)
    res = run_bass_kernel_spmd(nc, in_maps, list(range(NCORES)), trace=trace)
    return res


def assemble(results):
    h_emb = np.empty((N_TOK, D_HID), np.float32)
    h_out = np.empty((N_TOK, D_OUT), np.float32)
    imp = np.zeros((3, 8), np.float32)
    load = np.zeros((3, 8), np.float32)
    for c in range(NCORES):
        r = results[c]
        h_emb[c * N:(c + 1) * N] = r["hembT"].reshape(D_HID, N).T
        h_out[c * N:(c + 1) * N] = r["hT"].reshape(D_OUT, N).T
        il = r["il"].reshape(3, 16)
        imp += il[:, 0:8]
        load += il[:, 8:16]
    aux = np.float32(0.0)
    for li in range(3):
        aux += np.float32(LB_COEF) * (_cv_squared(imp[li]) + _cv_squared(load[li]))
    return h_emb, h_out, np.float32(aux)


def kernel(x=None, params=None, **kw):
    if x is None:
        x = kw["x"]
    if params is None:
        params = kw["params"]
    res = run_device(x, params, trace=False)
    return assemble(res.results)


# revision 26
# speedup vs baseline: 1.0119x; 1.0119x over previous
"""TRN2 Bass kernel for nn_MoE_19825569038534 (moe_routing).

3-layer MoE network: per layer a cosine router (top-4 of 8 experts,
softmax gate) + dense expert mix, LN+relu between layers.
Returns (h_emb, h, aux_loss) like the jax reference.

Strategy: data-parallel over tokens (8192 -> 8 cores x 1024). On-chip
activations are feature-major (hT [d, n]) so expert matmuls stream with
weights stationary and layers chain without activation transposes.
Expert sum uses gate PRE-scaling: out = sum_e (gate_e * h) @ W_e,
accumulated in PSUM/SBUF; gate rows are broadcast across partitions with
K=1 ones-matmuls.

Precision plan (routing top-k flips are the accuracy hazard: a flipped
token is ~0.6 absmax rel err): fp32 for anything that feeds a later
router (layers 0/1 experts, selection matmul h@ (P@s), LN stats),
float32r (full-speed, ~1.4e-4) for value-only paths (layer-2 experts,
router norm projection). Router temperature, sim normalization, P@s
fold, and the x2 layer temp are folded on the host.
"""

import numpy as np

import concourse.bacc as bacc
import concourse.mybir as mybir
from concourse import tile
from concourse.bass_utils import run_bass_kernel_spmd

F32 = mybir.dt.float32
F32R = mybir.dt.float32r
F16 = mybir.dt.float16
AF = mybir.ActivationFunctionType
OP = mybir.AluOpType

# ---- problem dims (hardcoded) ----
N_TOK = 8192
NCORES = 8
N = N_TOK // NCORES          # tokens per core
D_IN, D_HID, D_OUT = 512, 1024, 512
E = 8
RK = 4                       # router k = K+1
LOG100 = float(np.log(100.0))
LB_COEF = 0.01
LN_EPS = 1e-5
NB = N // 512                # 512-token free-dim blocks

# (di, do, has_ln, expert_mode)  mode: "f16x2" (fp32-grade, 3 passes),
# "f32r" (value-only precision, 1 pass), "f32" (native, 4 passes)
LAYERS = [
    (D_IN, D_HID, True, "f16x2"),
    (D_HID, D_HID, True, "f16x2"),
    (D_HID, D_OUT, False, "f16"),
]


def _build():
    nc = bacc.Bacc(None, target_bir_lowering=False)

    # ---------------- DRAM parameters ----------------
    xt_d = nc.declare_dram_parameter("xt", [D_IN // 128, 128, N], F32, isOutput=False)
    ident_d = nc.declare_dram_parameter("ident", [128, 128], F32, isOutput=False)
    sel_d = nc.declare_dram_parameter("sel", [E, E * 128], F32, isOutput=False)
    w_d, bf_d, ps_d, bs_d, pw_d, pb_d, gam_d, bet_d = {}, {}, {}, {}, {}, {}, {}, {}
    for li, (di, do, ln, ln_mode) in enumerate(LAYERS):
        KT, MT = di // 128, do // 128
        if ln_mode == "f16x2":
            w_d[li] = (
                nc.declare_dram_parameter(f"w{li}h", [E, MT, 128, KT, 128], F16, isOutput=False),
                nc.declare_dram_parameter(f"w{li}l", [E, MT, 128, KT, 128], F16, isOutput=False),
            )
        elif ln_mode == "f16":
            w_d[li] = nc.declare_dram_parameter(f"w{li}h", [E, MT, 128, KT, 128], F16, isOutput=False)
        else:
            w_d[li] = nc.declare_dram_parameter(f"w{li}", [E, MT, 128, KT, 128], F32, isOutput=False)
        bf_d[li] = nc.declare_dram_parameter(f"bf{li}", [E, do], F32, isOutput=False)
        ps_d[li] = nc.declare_dram_parameter(f"psf{li}", [128, KT, 8], F32, isOutput=False)
        bs_d[li] = nc.declare_dram_parameter(f"bsf{li}", [1, 8], F32, isOutput=False)
        pw_d[li] = nc.declare_dram_parameter(f"pw{li}", [D_HID // 128, 128, KT, 128], F16, isOutput=False)
        pb_d[li] = nc.declare_dram_parameter(f"pb{li}", [128, D_HID // 128], F32, isOutput=False)
        if ln:
            gam_d[li] = nc.declare_dram_parameter(f"gam{li}", [128, do // 128], F32, isOutput=False)
            bet_d[li] = nc.declare_dram_parameter(f"bet{li}", [128, do // 128], F32, isOutput=False)
    hemb_d = nc.declare_dram_parameter("hembT", [D_HID // 128, 128, N], F32, isOutput=True)
    ht_d = nc.declare_dram_parameter("hT", [D_OUT // 128, 128, N], F32, isOutput=True)
    il_d = nc.declare_dram_parameter("il", [1, 48], F32, isOutput=True)

    with tile.TileContext(nc) as tc:
        cpool = tc.alloc_tile_pool(name="consts", bufs=1)
        hpool = tc.alloc_tile_pool(name="hpool", bufs=1)
        wpool = tc.alloc_tile_pool(name="wpool", bufs=1)
        psum = tc.alloc_tile_pool(name="psum", bufs=1, space="PSUM")
        dpool = tc.alloc_tile_pool(name="dram", bufs=2, space="DRAM")

        # ---------------- constants ----------------
        ident = cpool.tile([128, 128], F32, tag="ident")
        nc.sync.dma_start(ident[:], ident_d[:])
        ones_row = cpool.tile([1, 128], F32, tag="ones_row")   # K=1 lhsT
        nc.vector.memset(ones_row[:], 1.0)
        ones_col = cpool.tile([128, 1], F32, tag="ones_col")   # partition-sum lhsT
        nc.vector.memset(ones_col[:], 1.0)
        ones_col_r = cpool.tile([128, 1], F32R, tag="ones_col_r")
        nc.vector.tensor_copy(ones_col_r[:], ones_col[:])
        sel = cpool.tile([E, E * 128], F32, tag="sel")         # row-e selector lhsT
        nc.sync.dma_start(sel[:], sel_d[:])
        eps1 = cpool.tile([1, 1], F32, tag="eps1")
        nc.vector.memset(eps1[:], LN_EPS)

        ps_sb, bs_sb, bf_sb, pb_sb, gam_sb, bet_sb = {}, {}, {}, {}, {}, {}
        for li, (di, do, ln, _) in enumerate(LAYERS):
            KT = di // 128
            ps_sb[li] = cpool.tile([128, KT, 8], F32, tag=f"psf{li}", name=f"psf{li}")
            nc.sync.dma_start(ps_sb[li][:], ps_d[li][:])
            bs_sb[li] = cpool.tile([1, 8], F32, tag=f"bsf{li}", name=f"bsf{li}")
            nc.sync.dma_start(bs_sb[li][:], bs_d[li][:])
            bf_sb[li] = cpool.tile([E, do], F32, tag=f"bff{li}", name=f"bff{li}")
            nc.sync.dma_start(bf_sb[li][:], bf_d[li][:])
            pb_sb[li] = cpool.tile([128, D_HID // 128], F32, tag=f"pbf{li}", name=f"pbf{li}")
            nc.sync.dma_start(pb_sb[li][:], pb_d[li][:])
            if ln:
                gam_sb[li] = cpool.tile([128, do // 128], F32, tag=f"gamf{li}", name=f"gamf{li}")
                nc.sync.dma_start(gam_sb[li][:], gam_d[li][:])
                bet_sb[li] = cpool.tile([128, do // 128], F32, tag=f"betf{li}", name=f"betf{li}")
                nc.sync.dma_start(bet_sb[li][:], bet_d[li][:])

        il_sb = cpool.tile([1, 48], F32, tag="il_sb")

        # ---------------- input ----------------
        h = []
        for k in range(D_IN // 128):
            t = hpool.tile([128, N], F32, tag="h", bufs=8)
            nc.sync.dma_start(t[:], xt_d[k])
            h.append(t)

        # ---------------- layers ----------------
        for li, (di, do, use_ln, emode) in enumerate(LAYERS):
            KT, MT = di // 128, do // 128
            PT = D_HID // 128  # proj width tiles (norm path)

            # --- norm projection: S2 = ||h @ P + pb||^2 per token (f16) ---
            s2row = wpool.tile([1, N], F32, tag="rowsb", bufs=4)
            for b in range(NB):
                sl = slice(b * 512, (b + 1) * 512)
                hrb = []
                for k in range(KT):
                    hr = wpool.tile([128, 512], F16, tag="hr", bufs=8)
                    nc.vector.tensor_copy(hr[:], h[k][:, sl])
                    hrb.append(hr)
                ps_s2 = psum.tile([1, 512], F32, tag="row", bufs=1)
                for m in range(PT):
                    pwm = wpool.tile([128, KT, 128], F16, tag="pwm", bufs=2)
                    nc.sync.dma_start(pwm[:], pw_d[li][m])
                    pt = psum.tile([128, 512], F32, tag="big", bufs=5)
                    for k in range(KT):
                        nc.tensor.matmul(pt[:], pwm[:, k, :],
                                         hrb[k][:], start=(k == 0), stop=(k == KT - 1))
                    sq = wpool.tile([128, 512], F32R, tag="sqr", bufs=2)
                    nc.scalar.activation(sq[:], pt[:], AF.Square, bias=pb_sb[li][:, m:m + 1])
                    nc.tensor.matmul(ps_s2[:], ones_col_r[:], sq[:],
                                     start=(m == 0), stop=(m == PT - 1))
                nc.vector.tensor_copy(s2row[:, sl], ps_s2[:])
            # --- selection numerator (token-major), fp32 ---
            numer = []
            for t in range(N // 128):
                pn = psum.tile([128, 8], F32, tag="small", bufs=2)
                nc.tensor.matmul(pn[:], ones_row[:], bs_sb[li][:], start=True, stop=False)
                for k in range(KT):
                    nc.tensor.matmul(pn[:], h[k][:, t * 128:(t + 1) * 128],
                                     ps_sb[li][:, k, :], start=False, stop=(k == KT - 1))
                ns = wpool.tile([128, 8], F32, tag="numer", bufs=8)
                nc.vector.tensor_copy(ns[:], pn[:])
                numer.append(ns)

            # keep-warm: tiny PE ops dep'd on the serial rs2 chain so HAM
            # stays at K=8/8 through the PE-idle router phase
            pkw = psum.tile([128, 8], F32, tag="small", bufs=2, name=f"pkw{li}")
            nc.tensor.matmul(pkw[:], ones_row[:], s2row[:, 0:8], start=True, stop=True)

            # rs2 = 1/sqrt(S2) = 1/||t|| (temp already folded into PSfold)
            scr = dpool.tile([1, N], F32, tag="scr")
            nc.sync.dma_start(scr[:], s2row[:])
            rs2a = wpool.tile([128, N // 128], F32, tag="rs2a", bufs=1)
            nc.sync.dma_start(rs2a[:], scr.rearrange("o (t p) -> (o p) t", p=128))
            rs2b = wpool.tile([128, N // 128], F32, tag="rs2b", bufs=1)
            nc.scalar.activation(rs2b[:], rs2a[:], AF.Sqrt)
            rs2 = wpool.tile([128, N // 128], F32, tag="rs2c", bufs=1)
            nc.vector.reciprocal(rs2[:], rs2b[:])
            nc.tensor.matmul(pkw[0:1, :], ones_col[:], rs2[:], start=True, stop=True)

            # --- top-4 mask, softmax gate (token-major) ---
            gate = []
            ps_il = psum.tile([1, 16], F32, tag="row", bufs=1)
            for t in range(N // 128):
                m8 = wpool.tile([128, 8], F32, tag="m8", bufs=2)
                nc.vector.max(m8[:], numer[t][:])
                pm = wpool.tile([128, 16], F32, tag="pm", bufs=2)
                nc.vector.tensor_scalar(pm[:, 8:16], numer[t][:], m8[:, 3:4], None, op0=OP.is_ge)
                se = wpool.tile([128, 1], F32, tag="se", bufs=2)
                nc.scalar.activation(pm[:, 0:8], numer[t][:], AF.Exp,
                                     scale=rs2[:, t:t + 1], accum_out=se[:])
                re = wpool.tile([128, 1], F32, tag="re", bufs=2)
                nc.vector.reciprocal(re[:], se[:])
                nc.vector.tensor_scalar_mul(pm[:, 0:8], pm[:, 0:8], re[:])
                g = wpool.tile([128, 8], F32, tag="gate", bufs=8)
                nc.vector.tensor_tensor(g[:], pm[:, 0:8], pm[:, 8:16], op=OP.mult)
                gate.append(g)
                nc.tensor.matmul(ps_il[:], ones_col[:], pm[:],
                                 start=(t == 0), stop=(t == N // 128 - 1))
                nc.tensor.matmul(pkw[0:1, :], ones_col[:], g[:], start=True, stop=True)
            nc.vector.tensor_copy(il_sb[:, li * 16:(li + 1) * 16], ps_il[:])

            # --- gate transpose: gT [E, N] ---
            gT = wpool.tile([E, N], F32, tag="gT", bufs=2)
            for t in range(N // 128):
                pgt = psum.tile([8, 128], F32, tag="small", bufs=2)
                nc.tensor.transpose(pgt[:], gate[t][:], ident[:])
                nc.vector.tensor_copy(gT[:, t * 128:(t + 1) * 128], pgt[:])

            # --- experts: out[o, t] = sum_e (gate_e h) @ W_e + gate @ bfold ---
            outs = [hpool.tile([128, N], F32, tag="out", bufs=8, name=f"out{_m}") for _m in range(MT)]

            def make_hg(e):
                """Allocate hg tiles; return (tiles, emission units) so the
                DVE work can be interleaved into the previous expert's m-loop
                (emitting it all upfront starves the psum->out adds)."""
                hgh, hgl = [], []
                for k in range(KT):
                    hgh.append(wpool.tile([128, N], F16, tag="hgh", bufs=16, name=f"hgh{k}"))
                    if emode == "f16x2":
                        hgl.append(wpool.tile([128, N], F16, tag="hgl", bufs=16, name=f"hgl{k}"))
                st = {}
                units = []
                for b in range(NB):
                    sl = slice(b * 512, (b + 1) * 512)

                    def u_bcast(b=b, sl=sl):
                        pbc = psum.tile([128, 512], F32, tag="big", bufs=5, name="pbc")
                        nc.tensor.matmul(pbc[:], sel[:, e * 128:(e + 1) * 128],
                                         gT[:, sl], start=True, stop=True)
                        bcs = wpool.tile([128, 512], F32, tag="bcs", bufs=2, name="bcs")
                        nc.vector.tensor_copy(bcs[:], pbc[:])
                        st[b] = bcs
                    units.append(u_bcast)
                    for k in range(KT):
                        def u_mul(b=b, sl=sl, k=k):
                            bcs = st[b]
                            if emode == "f16x2":
                                hgf = wpool.tile([128, 512], F32, tag="hgf", bufs=2, name="hgf")
                                nc.vector.tensor_tensor(hgf[:], h[k][:, sl], bcs[:], op=OP.mult)
                                nc.vector.tensor_copy(hgh[k][:, sl], hgf[:])
                                nc.vector.tensor_sub(hgl[k][:, sl], hgf[:], hgh[k][:, sl])
                            else:
                                nc.vector.tensor_tensor(hgh[k][:, sl], h[k][:, sl], bcs[:], op=OP.mult)
                        units.append(u_mul)
                return (hgh, hgl), units

            hg_cur, units0 = make_hg(0)
            for u in units0:
                u()
            pending = []
            for e in range(E):
                hgh, hgl = hg_cur
                if e + 1 < E:
                    hg_cur, pending = make_hg(e + 1)
                else:
                    pending = []
                for m in range(MT):
                    take = (len(pending) + MT - m - 1) // (MT - m)
                    for u in pending[:take]:
                        u()
                    pending = pending[take:]
                    if emode == "f16x2":
                        whi = wpool.tile([128, KT, 128], F16, tag="wh", bufs=3)
                        nc.sync.dma_start(whi[:], w_d[li][0][e, m])
                        wlo = wpool.tile([128, KT, 128], F16, tag="wl", bufs=2)
                        nc.sync.dma_start(wlo[:], w_d[li][1][e, m])
                    else:
                        whi = wpool.tile([128, KT, 128], F16, tag="wh", bufs=3)
                        nc.sync.dma_start(whi[:], w_d[li][e, m])
                    for b in range(NB):
                        sl = slice(b * 512, (b + 1) * 512)
                        po = psum.tile([128, 512], F32, tag="big", bufs=5)
                        first = True
                        if e == 0:
                            nc.tensor.matmul(po[:], bf_sb[li][:, m * 128:(m + 1) * 128],
                                             gT[:, sl], start=True, stop=False)
                            first = False
                        if emode == "f16x2":
                            passes = [(whi, hgh), (whi, hgl), (wlo, hgh)]
                        else:
                            passes = [(whi, hgh)]
                        np_ = len(passes)
                        for pi, (wt, ht) in enumerate(passes):
                            for k in range(KT):
                                nc.tensor.matmul(
                                    po[:], wt[:, k, :], ht[k][:, sl],
                                    start=(first and pi == 0 and k == 0),
                                    stop=(pi == np_ - 1 and k == KT - 1))
                        if e == 0:
                            nc.vector.tensor_copy(outs[m][:, sl], po[:])
                        else:
                            nc.vector.tensor_tensor(outs[m][:, sl], outs[m][:, sl], po[:], op=OP.add)

            # --- layer epilogue ---
            if li == 0:
                for m in range(MT):
                    nc.sync.dma_start(hemb_d[m], outs[m][:])
            if li == len(LAYERS) - 1:
                for m in range(MT):
                    nc.sync.dma_start(ht_d[m], outs[m][:])

            if use_ln:
                # S1/S2 over features (partitions), fp32
                s1row_ln = wpool.tile([1, N], F32, tag="rowsb", bufs=4)
                s2row_ln = wpool.tile([1, N], F32, tag="rowsb", bufs=4)
                for b in range(NB):
                    sl = slice(b * 512, (b + 1) * 512)
                    p1 = psum.tile([1, 512], F32, tag="row", bufs=1)
                    for m in range(MT):
                        nc.tensor.matmul(p1[:], ones_col[:], outs[m][:, sl],
                                         start=(m == 0), stop=(m == MT - 1))
                    nc.vector.tensor_copy(s1row_ln[:, sl], p1[:])
                for b in range(NB):
                    sl = slice(b * 512, (b + 1) * 512)
                    p2 = psum.tile([1, 512], F32, tag="row", bufs=1)
                    for m in range(MT):
                        sq = wpool.tile([128, 512], F32R, tag="sqr", bufs=2)
                        nc.scalar.activation(sq[:], outs[m][:, sl], AF.Square)
                        nc.tensor.matmul(p2[:], ones_col_r[:], sq[:],
                                         start=(m == 0), stop=(m == MT - 1))
                    nc.vector.tensor_copy(s2row_ln[:, sl], p2[:])
                mu = wpool.tile([1, N], F32, tag="rowsb", bufs=4)
                nc.vector.tensor_scalar_mul(mu[:], s1row_ln[:], 1.0 / do)
                v1 = wpool.tile([1, N], F32, tag="rowsb", bufs=4)
                nc.vector.tensor_scalar_mul(v1[:], s2row_ln[:], 1.0 / do)
                v2 = wpool.tile([1, N], F32, tag="rowsb", bufs=4)
                nc.vector.tensor_tensor(v2[:], mu[:], mu[:], op=OP.mult)
                var = wpool.tile([1, N], F32, tag="rowsb", bufs=4)
                nc.vector.tensor_sub(var[:], v1[:], v2[:])
                sd = wpool.tile([1, N], F32, tag="rowsb", bufs=4)
                nc.scalar.activation(sd[:], var[:], AF.Sqrt, bias=eps1[:])
                rstd = wpool.tile([1, N], F32, tag="rowsb", bufs=4)
                nc.vector.reciprocal(rstd[:], sd[:])
                nc.tensor.matmul(pkw[:], ones_row[:], mu[:, 0:8], start=True, stop=True)
                nc.tensor.matmul(pkw[:], ones_row[:], rstd[:, 0:8], start=True, stop=True)

                hn = [hpool.tile([128, N], F32, tag="h", bufs=8, name=f"hn{_m}") for _m in range(MT)]
                for b in range(NB):
                    sl = slice(b * 512, (b + 1) * 512)
                    pmu = psum.tile([128, 512], F32, tag="big", bufs=5)
                    nc.tensor.matmul(pmu[:], ones_row[:], mu[:, sl], start=True, stop=True)
                    mus = wpool.tile([128, 512], F32, tag="bcs", bufs=2)
                    nc.vector.tensor_copy(mus[:], pmu[:])
                    prs = psum.tile([128, 512], F32, tag="big", bufs=5)
                    nc.tensor.matmul(prs[:], ones_row[:], rstd[:, sl], start=True, stop=True)
                    rss = wpool.tile([128, 512], F32, tag="bcs", bufs=2)
                    nc.vector.tensor_copy(rss[:], prs[:])
                    for m in range(MT):
                        u = wpool.tile([128, 512], F32, tag="u", bufs=2)
                        nc.vector.tensor_sub(u[:], outs[m][:, sl], mus[:])
                        w2 = wpool.tile([128, 512], F32, tag="u", bufs=2)
                        nc.vector.tensor_tensor(w2[:], u[:], rss[:], op=OP.mult)
                        nc.scalar.activation(hn[m][:, sl], w2[:], AF.Relu,
                                             scale=gam_sb[li][:, m:m + 1],
                                             bias=bet_sb[li][:, m:m + 1])
                        nc.tensor.matmul(pkw[0:1, :], ones_col[:], hn[m][:, sl.start:sl.start + 8],
                                         start=True, stop=True)
                h = hn

        nc.sync.dma_start(il_d[:], il_sb[:])

        for _p in (dpool, psum, wpool, hpool, cpool):
            _p.release()

    nc.compile()
    return nc


_NC = None


def _get_nc():
    global _NC
    if _NC is None:
        _NC = _build()
    return _NC


def _np(a):
    return np.asarray(a, dtype=np.float32)


def host_prep(x, params):
    """Fold params on the host; build per-core in_maps."""
    x = _np(x)
    common = {"ident": np.eye(128, dtype=np.float32)}
    selm = np.zeros((E, E * 128), np.float32)
    for e in range(E):
        selm[e, e * 128:(e + 1) * 128] = 1.0
    common["sel"] = selm
    for li, (di, do, ln, _mode) in enumerate(LAYERS):
        lp = params["layers"][li]
        KT, MT = di // 128, do // 128
        proj_W = np.asarray(lp["proj_W"], np.float64)
        proj_b = np.asarray(lp["proj_b"], np.float64)
        sim = np.asarray(lp["sim"], np.float64)
        rtemp = float(np.exp(min(float(np.asarray(lp["router_temp"]).reshape(-1)[0]), LOG100)))
        ltemp = float(np.exp(min(float(np.asarray(lp["layer_temp"]).reshape(-1)[0]), LOG100)))
        s = sim / np.maximum(np.linalg.norm(sim, axis=0, keepdims=True), 1e-12)
        PS = (proj_W @ s) * rtemp                      # [di, 8]
        bS = (proj_b @ s) * rtemp                      # [8]
        Wf = _np(lp["expert_W"]) * np.float32(ltemp)   # [E, di, do]
        bf = _np(lp["expert_b"]) * np.float32(ltemp)   # [E, do]
        # w layout [E, MT, 128(di_p), KT, 128(do_q)]
        w = np.ascontiguousarray(
            Wf.reshape(E, KT, 128, MT, 128).transpose(0, 3, 2, 1, 4))
        if LAYERS[li][3] == "f16x2":
            whi = w.astype(np.float16)
            wlo = (w - whi.astype(np.float32)).astype(np.float16)
            common[f"w{li}h"] = whi
            common[f"w{li}l"] = wlo
        elif LAYERS[li][3] == "f16":
            common[f"w{li}h"] = w.astype(np.float16)
        else:
            common[f"w{li}"] = w
        common[f"bf{li}"] = np.ascontiguousarray(bf)
        common[f"psf{li}"] = np.ascontiguousarray(
            PS.astype(np.float32).reshape(KT, 128, 8).transpose(1, 0, 2))
        common[f"bsf{li}"] = bS.astype(np.float32).reshape(1, 8)
        common[f"pw{li}"] = np.ascontiguousarray(
            _np(lp["proj_W"]).reshape(KT, 128, D_HID // 128, 128)
            .transpose(2, 1, 0, 3)).astype(np.float16)
        common[f"pb{li}"] = np.ascontiguousarray(
            _np(lp["proj_b"]).reshape(D_HID // 128, 128).T)
        if ln:
            lnp = params["ln"][li]
            common[f"gam{li}"] = np.ascontiguousarray(_np(lnp["scale"]).reshape(MT, 128).T)
            common[f"bet{li}"] = np.ascontiguousarray(_np(lnp["bias"]).reshape(MT, 128).T)

    in_maps = []
    for c in range(NCORES):
        xs = x[c * N:(c + 1) * N]                      # [N, D_IN]
        xt = np.ascontiguousarray(xs.T.reshape(D_IN // 128, 128, N))
        m = dict(common)
        m["xt"] = xt
        in_maps.append(m)
    return in_maps


def _cv_squared(v):
    v = np.asarray(v, np.float32)
    return np.var(v, ddof=1) / (np.mean(v) ** 2 + np.float32(1e-10))


def run_device(x, params, trace=False):
    nc = _get_nc()
    in_maps = host_prep(x, params)
    res = run_bass_kernel_spmd(nc, in_maps, list(range(NCORES)), trace=trace)
    return res


def assemble(results):
    h_emb = np.empty((N_TOK, D_HID), np.float32)
    h_out = np.empty((N_TOK, D_OUT), np.float32)
    imp = np.zeros((3, 8), np.float32)
    load = np.zeros((3, 8), np.float32)
    for c in range(NCORES):
        r = results[c]
        h_emb[c * N:(c + 1) * N] = r["hembT"].reshape(D_HID, N).T
        h_out[c * N:(c + 1) * N] = r["hT"].reshape(D_OUT, N).T
        il = r["il"].reshape(3, 16)
        imp += il[:, 0:8]
        load += il[:, 8:16]
    aux = np.float32(0.0)
    for li in range(3):
        aux += np.float32(LB_COEF) * (_cv_squared(imp[li]) + _cv_squared(load[li]))
    return h_emb, h_out, np.float32(aux)


def kernel(x=None, params=None, **kw):
    if x is None:
        x = kw["x"]
    if params is None:
        params = kw["params"]
    res = run_device(x, params, trace=False)
    return assemble(res.results)


# revision 27
# speedup vs baseline: 1.0145x; 1.0025x over previous
"""TRN2 Bass kernel for nn_MoE_19825569038534 (moe_routing).

3-layer MoE network: per layer a cosine router (top-4 of 8 experts,
softmax gate) + dense expert mix, LN+relu between layers.
Returns (h_emb, h, aux_loss) like the jax reference.

Strategy: data-parallel over tokens (8192 -> 8 cores x 1024). On-chip
activations are feature-major (hT [d, n]) so expert matmuls stream with
weights stationary and layers chain without activation transposes.
Expert sum uses gate PRE-scaling: out = sum_e (gate_e * h) @ W_e,
accumulated in PSUM per expert then SBUF across experts; gate rows are
broadcast across partitions with selector/ones matmuls. The next
expert's gate-scaled activations are produced in small units interleaved
through the current expert's loop (per-engine program order!), and tiny
keep-warm matmuls chained on the serial router/LN paths hold the PE
clock-gate at full speed.

Precision plan (routing top-k flips are the accuracy hazard: a flipped
token is ~0.6 absmax rel err): f16 hi/lo 3-pass split matmuls
(fp32-grade, measured 1.9e-7) for layers 0/1 experts; fp32 for the
selection matmul h@(P@s) and LN mean; f32r for variance/S2 stats
(ln.bias==0 makes rstd error a flip-safe per-token scale); single-pass
f16 for value-only paths (layer-2 experts, router norm projection).
Router temperature, sim normalization, P@s fold, and the x2 layer temp
are folded on the host.
"""

import numpy as np

import concourse.bacc as bacc
import concourse.mybir as mybir
from concourse import tile
from concourse.bass_utils import run_bass_kernel_spmd

F32 = mybir.dt.float32
F32R = mybir.dt.float32r
F16 = mybir.dt.float16
AF = mybir.ActivationFunctionType
OP = mybir.AluOpType

# ---- problem dims (hardcoded) ----
N_TOK = 8192
NCORES = 8
N = N_TOK // NCORES          # tokens per core
D_IN, D_HID, D_OUT = 512, 1024, 512
E = 8
RK = 4                       # router k = K+1
LOG100 = float(np.log(100.0))
LB_COEF = 0.01
LN_EPS = 1e-5
NB = N // 512                # 512-token free-dim blocks

# (di, do, has_ln, expert_mode)  mode: "f16x2" (fp32-grade, 3 passes),
# "f32r" (value-only precision, 1 pass), "f32" (native, 4 passes)
LAYERS = [
    (D_IN, D_HID, True, "f16x2"),
    (D_HID, D_HID, True, "f16x2"),
    (D_HID, D_OUT, False, "f16"),
]


def _build():
    nc = bacc.Bacc(None, target_bir_lowering=False)

    # ---------------- DRAM parameters ----------------
    xt_d = nc.declare_dram_parameter("xt", [D_IN // 128, 128, N], F32, isOutput=False)
    ident_d = nc.declare_dram_parameter("ident", [128, 128], F32, isOutput=False)
    sel_d = nc.declare_dram_parameter("sel", [E, E * 128], F32, isOutput=False)
    w_d, bf_d, ps_d, bs_d, pw_d, pb_d, gam_d, bet_d = {}, {}, {}, {}, {}, {}, {}, {}
    for li, (di, do, ln, ln_mode) in enumerate(LAYERS):
        KT, MT = di // 128, do // 128
        if ln_mode == "f16x2":
            w_d[li] = (
                nc.declare_dram_parameter(f"w{li}h", [E, MT, 128, KT, 128], F16, isOutput=False),
                nc.declare_dram_parameter(f"w{li}l", [E, MT, 128, KT, 128], F16, isOutput=False),
            )
        elif ln_mode == "f16":
            w_d[li] = nc.declare_dram_parameter(f"w{li}h", [E, MT, 128, KT, 128], F16, isOutput=False)
        else:
            w_d[li] = nc.declare_dram_parameter(f"w{li}", [E, MT, 128, KT, 128], F32, isOutput=False)
        bf_d[li] = nc.declare_dram_parameter(f"bf{li}", [E, do], F32, isOutput=False)
        ps_d[li] = nc.declare_dram_parameter(f"psf{li}", [128, KT, 8], F32, isOutput=False)
        bs_d[li] = nc.declare_dram_parameter(f"bsf{li}", [1, 8], F32, isOutput=False)
        pw_d[li] = nc.declare_dram_parameter(f"pw{li}", [D_HID // 128, 128, KT, 128], F16, isOutput=False)
        pb_d[li] = nc.declare_dram_parameter(f"pb{li}", [128, D_HID // 128], F32, isOutput=False)
        if ln:
            gam_d[li] = nc.declare_dram_parameter(f"gam{li}", [128, do // 128], F32, isOutput=False)
            bet_d[li] = nc.declare_dram_parameter(f"bet{li}", [128, do // 128], F32, isOutput=False)
    hemb_d = nc.declare_dram_parameter("hembT", [D_HID // 128, 128, N], F32, isOutput=True)
    ht_d = nc.declare_dram_parameter("hT", [D_OUT // 128, 128, N], F32, isOutput=True)
    il_d = nc.declare_dram_parameter("il", [1, 48], F32, isOutput=True)

    with tile.TileContext(nc) as tc:
        cpool = tc.alloc_tile_pool(name="consts", bufs=1)
        hpool = tc.alloc_tile_pool(name="hpool", bufs=1)
        wpool = tc.alloc_tile_pool(name="wpool", bufs=1)
        psum = tc.alloc_tile_pool(name="psum", bufs=1, space="PSUM")
        dpool = tc.alloc_tile_pool(name="dram", bufs=2, space="DRAM")

        # ---------------- constants ----------------
        ident = cpool.tile([128, 128], F32, tag="ident")
        nc.sync.dma_start(ident[:], ident_d[:])
        ones_row = cpool.tile([1, 128], F32, tag="ones_row")   # K=1 lhsT
        nc.vector.memset(ones_row[:], 1.0)
        ones_col = cpool.tile([128, 1], F32, tag="ones_col")   # partition-sum lhsT
        nc.vector.memset(ones_col[:], 1.0)
        ones_col_r = cpool.tile([128, 1], F32R, tag="ones_col_r")
        nc.vector.tensor_copy(ones_col_r[:], ones_col[:])
        sel = cpool.tile([E, E * 128], F32, tag="sel")         # row-e selector lhsT
        nc.sync.dma_start(sel[:], sel_d[:])
        eps1 = cpool.tile([1, 1], F32, tag="eps1")
        nc.vector.memset(eps1[:], LN_EPS)

        ps_sb, bs_sb, bf_sb, pb_sb, gam_sb, bet_sb = {}, {}, {}, {}, {}, {}
        for li, (di, do, ln, _) in enumerate(LAYERS):
            KT = di // 128
            ps_sb[li] = cpool.tile([128, KT, 8], F32, tag=f"psf{li}", name=f"psf{li}")
            nc.sync.dma_start(ps_sb[li][:], ps_d[li][:])
            bs_sb[li] = cpool.tile([1, 8], F32, tag=f"bsf{li}", name=f"bsf{li}")
            nc.sync.dma_start(bs_sb[li][:], bs_d[li][:])
            bf_sb[li] = cpool.tile([E, do], F32, tag=f"bff{li}", name=f"bff{li}")
            nc.sync.dma_start(bf_sb[li][:], bf_d[li][:])
            pb_sb[li] = cpool.tile([128, D_HID // 128], F32, tag=f"pbf{li}", name=f"pbf{li}")
            nc.sync.dma_start(pb_sb[li][:], pb_d[li][:])
            if ln:
                gam_sb[li] = cpool.tile([128, do // 128], F32, tag=f"gamf{li}", name=f"gamf{li}")
                nc.sync.dma_start(gam_sb[li][:], gam_d[li][:])
                bet_sb[li] = cpool.tile([128, do // 128], F32, tag=f"betf{li}", name=f"betf{li}")
                nc.sync.dma_start(bet_sb[li][:], bet_d[li][:])

        il_sb = cpool.tile([1, 48], F32, tag="il_sb")

        # ---------------- input ----------------
        h = []
        for k in range(D_IN // 128):
            t = hpool.tile([128, N], F32, tag="h", bufs=8)
            nc.sync.dma_start(t[:], xt_d[k])
            h.append(t)

        # ---------------- layers ----------------
        for li, (di, do, use_ln, emode) in enumerate(LAYERS):
            KT, MT = di // 128, do // 128
            PT = D_HID // 128  # proj width tiles (norm path)

            # --- norm projection: S2 = ||h @ P + pb||^2 per token (f16) ---
            s2row = wpool.tile([1, N], F32, tag="rowsb", bufs=4)
            for b in range(NB):
                sl = slice(b * 512, (b + 1) * 512)
                hrb = []
                for k in range(KT):
                    hr = wpool.tile([128, 512], F16, tag="hr", bufs=8)
                    nc.vector.tensor_copy(hr[:], h[k][:, sl])
                    hrb.append(hr)
                ps_s2 = psum.tile([1, 512], F32, tag="row", bufs=1)
                for m in range(PT):
                    pwm = wpool.tile([128, KT, 128], F16, tag="pwm", bufs=2)
                    nc.sync.dma_start(pwm[:], pw_d[li][m])
                    pt = psum.tile([128, 512], F32, tag="big", bufs=5)
                    for k in range(KT):
                        nc.tensor.matmul(pt[:], pwm[:, k, :],
                                         hrb[k][:], start=(k == 0), stop=(k == KT - 1))
                    sq = wpool.tile([128, 512], F32R, tag="sqr", bufs=2)
                    nc.scalar.activation(sq[:], pt[:], AF.Square, bias=pb_sb[li][:, m:m + 1])
                    nc.tensor.matmul(ps_s2[:], ones_col_r[:], sq[:],
                                     start=(m == 0), stop=(m == PT - 1))
                nc.vector.tensor_copy(s2row[:, sl], ps_s2[:])
            # --- selection numerator (token-major), fp32 ---
            numer = []
            for t in range(N // 128):
                pn = psum.tile([128, 8], F32, tag="small", bufs=2)
                nc.tensor.matmul(pn[:], ones_row[:], bs_sb[li][:], start=True, stop=False)
                for k in range(KT):
                    nc.tensor.matmul(pn[:], h[k][:, t * 128:(t + 1) * 128],
                                     ps_sb[li][:, k, :], start=False, stop=(k == KT - 1))
                ns = wpool.tile([128, 8], F32, tag="numer", bufs=8)
                nc.vector.tensor_copy(ns[:], pn[:])
                numer.append(ns)

            # keep-warm: tiny PE ops dep'd on the serial rs2 chain so HAM
            # stays at K=8/8 through the PE-idle router phase
            pkw = psum.tile([128, 8], F32, tag="small", bufs=2, name=f"pkw{li}")
            nc.tensor.matmul(pkw[:], ones_row[:], s2row[:, 0:8], start=True, stop=True)

            # rs2 = 1/sqrt(S2) = 1/||t|| (temp already folded into PSfold)
            scr = dpool.tile([1, N], F32, tag="scr")
            nc.sync.dma_start(scr[:], s2row[:])
            rs2a = wpool.tile([128, N // 128], F32, tag="rs2a", bufs=1)
            nc.sync.dma_start(rs2a[:], scr.rearrange("o (t p) -> (o p) t", p=128))
            rs2b = wpool.tile([128, N // 128], F32, tag="rs2b", bufs=1)
            nc.scalar.activation(rs2b[:], rs2a[:], AF.Sqrt)
            rs2 = wpool.tile([128, N // 128], F32, tag="rs2c", bufs=1)
            nc.vector.reciprocal(rs2[:], rs2b[:])
            nc.tensor.matmul(pkw[0:1, :], ones_col[:], rs2[:], start=True, stop=True)

            # --- top-4 mask, softmax gate (token-major) ---
            gate = []
            ps_il = psum.tile([1, 16], F32, tag="row", bufs=1)
            for t in range(N // 128):
                m8 = wpool.tile([128, 8], F32, tag="m8", bufs=2)
                nc.vector.max(m8[:], numer[t][:])
                pm = wpool.tile([128, 16], F32, tag="pm", bufs=2)
                nc.vector.tensor_scalar(pm[:, 8:16], numer[t][:], m8[:, 3:4], None, op0=OP.is_ge)
                se = wpool.tile([128, 1], F32, tag="se", bufs=2)
                nc.scalar.activation(pm[:, 0:8], numer[t][:], AF.Exp,
                                     scale=rs2[:, t:t + 1], accum_out=se[:])
                re = wpool.tile([128, 1], F32, tag="re", bufs=2)
                nc.vector.reciprocal(re[:], se[:])
                nc.vector.tensor_scalar_mul(pm[:, 0:8], pm[:, 0:8], re[:])
                g = wpool.tile([128, 8], F32, tag="gate", bufs=8)
                nc.vector.tensor_tensor(g[:], pm[:, 0:8], pm[:, 8:16], op=OP.mult)
                gate.append(g)
                nc.tensor.matmul(ps_il[:], ones_col[:], pm[:],
                                 start=(t == 0), stop=(t == N // 128 - 1))
                nc.tensor.matmul(pkw[0:1, :], ones_col[:], g[:], start=True, stop=True)
            nc.vector.tensor_copy(il_sb[:, li * 16:(li + 1) * 16], ps_il[:])

            # --- gate transpose: gT [E, N] ---
            gT = wpool.tile([E, N], F32, tag="gT", bufs=2)
            for t in range(N // 128):
                pgt = psum.tile([8, 128], F32, tag="small", bufs=2)
                nc.tensor.transpose(pgt[:], gate[t][:], ident[:])
                nc.vector.tensor_copy(gT[:, t * 128:(t + 1) * 128], pgt[:])

            # --- experts: out[o, t] = sum_e (gate_e h) @ W_e + gate @ bfold ---
            outs = [hpool.tile([128, N], F32, tag="out", bufs=8, name=f"out{_m}") for _m in range(MT)]

            def make_hg(e):
                """Allocate hg tiles; return (tiles, emission units) so the
                DVE work can be interleaved into the previous expert's m-loop
                (emitting it all upfront starves the psum->out adds)."""
                hgh, hgl = [], []
                for k in range(KT):
                    hgh.append(wpool.tile([128, N], F16, tag="hgh", bufs=16, name=f"hgh{k}"))
                    if emode == "f16x2":
                        hgl.append(wpool.tile([128, N], F16, tag="hgl", bufs=16, name=f"hgl{k}"))
                st = {}
                units = []
                for b in range(NB):
                    sl = slice(b * 512, (b + 1) * 512)

                    def u_bcast(b=b, sl=sl):
                        pbc = psum.tile([128, 512], F32, tag="big", bufs=5, name="pbc")
                        nc.tensor.matmul(pbc[:], sel[:, e * 128:(e + 1) * 128],
                                         gT[:, sl], start=True, stop=True)
                        bcs = wpool.tile([128, 512], F32, tag="bcs", bufs=2, name="bcs")
                        nc.vector.tensor_copy(bcs[:], pbc[:])
                        st[b] = bcs
                    units.append(u_bcast)
                    for k in range(KT):
                        def u_mul(b=b, sl=sl, k=k):
                            bcs = st[b]
                            if emode == "f16x2":
                                hgf = wpool.tile([128, 512], F32, tag="hgf", bufs=2, name="hgf")
                                nc.vector.tensor_tensor(hgf[:], h[k][:, sl], bcs[:], op=OP.mult)
                                nc.vector.tensor_copy(hgh[k][:, sl], hgf[:])
                                nc.vector.tensor_sub(hgl[k][:, sl], hgf[:], hgh[k][:, sl])
                            else:
                                nc.vector.tensor_tensor(hgh[k][:, sl], h[k][:, sl], bcs[:], op=OP.mult)
                        units.append(u_mul)
                return (hgh, hgl), units

            hg_cur, units0 = make_hg(0)
            for u in units0:
                u()
            pending = []
            for e in range(E):
                hgh, hgl = hg_cur
                if e + 1 < E:
                    hg_cur, pending = make_hg(e + 1)
                else:
                    pending = []
                for m in range(MT):
                    take = (len(pending) + MT - m - 1) // (MT - m)
                    for u in pending[:take]:
                        u()
                    pending = pending[take:]
                    if emode == "f16x2":
                        whi = wpool.tile([128, KT, 128], F16, tag="wh", bufs=3)
                        nc.sync.dma_start(whi[:], w_d[li][0][e, m])
                        wlo = wpool.tile([128, KT, 128], F16, tag="wl", bufs=2)
                        nc.sync.dma_start(wlo[:], w_d[li][1][e, m])
                    else:
                        whi = wpool.tile([128, KT, 128], F16, tag="wh", bufs=3)
                        nc.sync.dma_start(whi[:], w_d[li][e, m])
                    for b in range(NB):
                        sl = slice(b * 512, (b + 1) * 512)
                        po = psum.tile([128, 512], F32, tag="big", bufs=5)
                        first = True
                        if e == 0:
                            nc.tensor.matmul(po[:], bf_sb[li][:, m * 128:(m + 1) * 128],
                                             gT[:, sl], start=True, stop=False)
                            first = False
                        if emode == "f16x2":
                            passes = [(whi, hgh), (whi, hgl), (wlo, hgh)]
                        else:
                            passes = [(whi, hgh)]
                        np_ = len(passes)
                        for pi, (wt, ht) in enumerate(passes):
                            for k in range(KT):
                                nc.tensor.matmul(
                                    po[:], wt[:, k, :], ht[k][:, sl],
                                    start=(first and pi == 0 and k == 0),
                                    stop=(pi == np_ - 1 and k == KT - 1))
                        if e == 0:
                            nc.vector.tensor_copy(outs[m][:, sl], po[:])
                        else:
                            nc.vector.tensor_tensor(outs[m][:, sl], outs[m][:, sl], po[:], op=OP.add)

            # --- layer epilogue ---
            if li == 0:
                for m in range(MT):
                    nc.sync.dma_start(hemb_d[m], outs[m][:])
            if li == len(LAYERS) - 1:
                for m in range(MT):
                    nc.sync.dma_start(ht_d[m], outs[m][:])

            if use_ln:
                # S1/S2 over features (partitions), fp32
                s1row_ln = wpool.tile([1, N], F32, tag="rowsb", bufs=4)
                s2row_ln = wpool.tile([1, N], F32, tag="rowsb", bufs=4)
                for b in range(NB):
                    sl = slice(b * 512, (b + 1) * 512)
                    p1 = psum.tile([1, 512], F32, tag="row", bufs=1)
                    for m in range(MT):
                        nc.tensor.matmul(p1[:], ones_col[:], outs[m][:, sl],
                                         start=(m == 0), stop=(m == MT - 1))
                    nc.vector.tensor_copy(s1row_ln[:, sl], p1[:])
                for b in range(NB):
                    sl = slice(b * 512, (b + 1) * 512)
                    p2 = psum.tile([1, 512], F32, tag="row", bufs=1)
                    for m in range(MT):
                        sq = wpool.tile([128, 512], F32R, tag="sqr", bufs=2)
                        nc.scalar.activation(sq[:], outs[m][:, sl], AF.Square)
                        nc.tensor.matmul(p2[:], ones_col_r[:], sq[:],
                                         start=(m == 0), stop=(m == MT - 1))
                    nc.vector.tensor_copy(s2row_ln[:, sl], p2[:])
                mu = wpool.tile([1, N], F32, tag="rowsb", bufs=4)
                nc.vector.tensor_scalar_mul(mu[:], s1row_ln[:], 1.0 / do)
                v1 = wpool.tile([1, N], F32, tag="rowsb", bufs=4)
                nc.vector.tensor_scalar_mul(v1[:], s2row_ln[:], 1.0 / do)
                v2 = wpool.tile([1, N], F32, tag="rowsb", bufs=4)
                nc.vector.tensor_tensor(v2[:], mu[:], mu[:], op=OP.mult)
                var = wpool.tile([1, N], F32, tag="rowsb", bufs=4)
                nc.vector.tensor_sub(var[:], v1[:], v2[:])
                sd = wpool.tile([1, N], F32, tag="rowsb", bufs=4)
                nc.scalar.activation(sd[:], var[:], AF.Sqrt, bias=eps1[:])
                rstd = wpool.tile([1, N], F32, tag="rowsb", bufs=4)
                nc.vector.reciprocal(rstd[:], sd[:])
                nc.tensor.matmul(pkw[:], ones_row[:], mu[:, 0:8], start=True, stop=True)
                nc.tensor.matmul(pkw[:], ones_row[:], rstd[:, 0:8], start=True, stop=True)

                hn = [hpool.tile([128, N], F32, tag="h", bufs=8, name=f"hn{_m}") for _m in range(MT)]
                for b in range(NB):
                    sl = slice(b * 512, (b + 1) * 512)
                    pmu = psum.tile([128, 512], F32, tag="big", bufs=5)
                    nc.tensor.matmul(pmu[:], ones_row[:], mu[:, sl], start=True, stop=True)
                    mus = wpool.tile([128, 512], F32, tag="bcs", bufs=2)
                    nc.vector.tensor_copy(mus[:], pmu[:])
                    prs = psum.tile([128, 512], F32, tag="big", bufs=5)
                    nc.tensor.matmul(prs[:], ones_row[:], rstd[:, sl], start=True, stop=True)
                    rss = wpool.tile([128, 512], F32, tag="bcs", bufs=2)
                    nc.vector.tensor_copy(rss[:], prs[:])
                    for m in range(MT):
                        u = wpool.tile([128, 512], F32, tag="u", bufs=2)
                        nc.vector.tensor_sub(u[:], outs[m][:, sl], mus[:])
                        w2 = wpool.tile([128, 512], F32, tag="u", bufs=2)
                        nc.vector.tensor_tensor(w2[:], u[:], rss[:], op=OP.mult)
                        nc.scalar.activation(hn[m][:, sl], w2[:], AF.Relu,
                                             scale=gam_sb[li][:, m:m + 1],
                                             bias=bet_sb[li][:, m:m + 1])
                        nc.tensor.matmul(pkw[0:1, :], ones_col[:], hn[m][:, sl.start:sl.start + 8],
                                         start=True, stop=True)
                h = hn

        nc.sync.dma_start(il_d[:], il_sb[:])

        for _p in (dpool, psum, wpool, hpool, cpool):
            _p.release()

    nc.compile()
    return nc


_NC = None


def _get_nc():
    global _NC
    if _NC is None:
        _NC = _build()
    return _NC


def _np(a):
    return np.asarray(a, dtype=np.float32)


def host_prep(x, params):
    """Fold params on the host; build per-core in_maps."""
    x = _np(x)
    common = {"ident": np.eye(128, dtype=np.float32)}
    selm = np.zeros((E, E * 128), np.float32)
    for e in range(E):
        selm[e, e * 128:(e + 1) * 128] = 1.0
    common["sel"] = selm
    for li, (di, do, ln, _mode) in enumerate(LAYERS):
        lp = params["layers"][li]
        KT, MT = di // 128, do // 128
        proj_W = np.asarray(lp["proj_W"], np.float64)
        proj_b = np.asarray(lp["proj_b"], np.float64)
        sim = np.asarray(lp["sim"], np.float64)
        rtemp = float(np.exp(min(float(np.asarray(lp["router_temp"]).reshape(-1)[0]), LOG100)))
        ltemp = float(np.exp(min(float(np.asarray(lp["layer_temp"]).reshape(-1)[0]), LOG100)))
        s = sim / np.maximum(np.linalg.norm(sim, axis=0, keepdims=True), 1e-12)
        PS = (proj_W @ s) * rtemp                      # [di, 8]
        bS = (proj_b @ s) * rtemp                      # [8]
        Wf = _np(lp["expert_W"]) * np.float32(ltemp)   # [E, di, do]
        bf = _np(lp["expert_b"]) * np.float32(ltemp)   # [E, do]
        # w layout [E, MT, 128(di_p), KT, 128(do_q)]
        w = np.ascontiguousarray(
            Wf.reshape(E, KT, 128, MT, 128).transpose(0, 3, 2, 1, 4))
        if LAYERS[li][3] == "f16x2":
            whi = w.astype(np.float16)
            wlo = (w - whi.astype(np.float32)).astype(np.float16)
            common[f"w{li}h"] = whi
            common[f"w{li}l"] = wlo
        elif LAYERS[li][3] == "f16":
            common[f"w{li}h"] = w.astype(np.float16)
        else:
            common[f"w{li}"] = w
        common[f"bf{li}"] = np.ascontiguousarray(bf)
        common[f"psf{li}"] = np.ascontiguousarray(
            PS.astype(np.float32).reshape(KT, 128, 8).transpose(1, 0, 2))
        common[f"bsf{li}"] = bS.astype(np.float32).reshape(1, 8)
        common[f"pw{li}"] = np.ascontiguousarray(
            _np(lp["proj_W"]).reshape(KT, 128, D_HID // 128, 128)
            .transpose(2, 1, 0, 3)).astype(np.float16)
        common[f"pb{li}"] = np.ascontiguousarray(
            _np(lp["proj_b"]).reshape(D_HID // 128, 128).T)
        if ln:
            lnp = params["ln"][li]
            common[f"gam{li}"] = np.ascontiguousarray(_np(lnp["scale"]).reshape(MT, 128).T)
            common[f"bet{li}"] = np.ascontiguousarray(_np(lnp["bias"]).reshape(MT, 128).T)

    in_maps = []
    for c in range(NCORES):
        xs = x[c * N:(c + 1) * N]                      # [N, D_IN]
        xt = np.ascontiguousarray(xs.T.reshape(D_IN // 128, 128, N))
        m = dict(common)
        m["xt"] = xt
        in_maps.append(m)
    return in_maps


def _cv_squared(v):
    v = np.asarray(v, np.float32)
    return np.var(v, ddof=1) / (np.mean(v) ** 2 + np.float32(1e-10))


def run_device(x, params, trace=False):
    nc = _get_nc()
    in_maps = host_prep(x, params)
    res = run_bass_kernel_spmd(nc, in_maps, list(range(NCORES)), trace=trace)
    return res


def assemble(results):
    h_emb = np.empty((N_TOK, D_HID), np.float32)
    h_out = np.empty((N_TOK, D_OUT), np.float32)
    imp = np.zeros((3, 8), np.float32)
    load = np.zeros((3, 8), np.float32)
    for c in range(NCORES):
        r = results[c]
        h_emb[c * N:(c + 1) * N] = r["hembT"].reshape(D_HID, N).T
        h_out[c * N:(c + 1) * N] = r["hT"].reshape(D_OUT, N).T
        il = r["il"].reshape(3, 16)
        imp += il[:, 0:8]
        load += il[:, 8:16]
    aux = np.float32(0.0)
    for li in range(3):
        aux += np.float32(LB_COEF) * (_cv_squared(imp[li]) + _cv_squared(load[li]))
    return h_emb, h_out, np.float32(aux)


def kernel(x=None, params=None, **kw):
    if x is None:
        x = kw["x"]
    if params is None:
        params = kw["params"]
    res = run_device(x, params, trace=False)
    return assemble(res.results)
